# revision 19
# baseline (speedup 1.0000x reference)
"""Trainium2 Bass kernel for nn_EquivariantLayer — bf16 redesign.

Data-parallel over batch (2 samples/core x 8 cores). All DFTs are matmuls
on the TensorEngine in bf16 (1 cyc/row vs 4 for f32). Cross products on
DVE/Pool in bf16 (2x DVE mode). Output staged per-sample in SBUF (bf16)
and stored with 3 chunk-DMAs per sample over the 3 DMA queues
(SP / Activation / Pool). Host casts bf16 -> f32.

Per sample:
  fsb    = cast-load f (Pool SWDGE, f32->bf16)      [64, (i8,y64)]
  psA    = fsb^T @ ExF    (x-DFT)                   [128, (RI,kx) 192]
  psT1   = fsb^T @ RxT    (fr row transform)        [128, X 128]
  Fcv    = y-DFT (EyC/EyS accumulate)               [128, (h,RI,kx) 256]
  Mw[RI] = Fcv (x) k_sb   (conv products)           [128, 2048]
  ps_acv = S_sel @ Mw     (i-reduction)             [32, 512] x4
  Bu/Bv  = acv (x) tsg    (uncurl multipliers)      [32, 1024] x4
  psG    = B @ QF1 + B_I @ QF2  (ky-inverse)        [128=(ch2,kx), (RI,Y) 256]
  Gsb    = drain psG (1 copy)
  psU    = PRT64 @ G_R - PIT64 @ G_I per ch (kx-inverse, accumulated
           into column blocks, 4 ch per PSUM bank)  [128, 512]
  u_s/v_s fields bf16; fr direct path via CyT
  W      = u_a * v_b all 16x16 ordered products     [128, 32768] bf16
  subs   -> staging tiles st0/st1/st2 (bf16), ch-grouped
  DMA    st* -> out_sh[b] on SP/ACT/POOL queues
"""
import sys
import numpy as np
import ml_dtypes

if '/opt/trn_rl_repo' not in sys.path:
    sys.path.insert(0, '/opt/trn_rl_repo')

import concourse.bass as bass
from concourse import bacc
import concourse.mybir as mybir
import concourse.tile as tile
from concourse.bass import AP
from concourse.bass_utils import run_bass_kernel_spmd

F32 = mybir.dt.float32
BF16 = mybir.dt.bfloat16
N_CORES = 8
B_PER_CORE = 2
C1, C2, N1, N2 = 8, 16, 64, 128
NCH_OUT = 128

PAIR_BASE = {}
_p = 8
for _a in range(15):
    PAIR_BASE[_a] = _p
    _p += 15 - _a
assert _p == 128

CH_SPLITS = [0, 37, 62, 83, 100, 113, 128]  # a-run aligned chunk bounds


def _bf16(a):
    return np.ascontiguousarray(np.asarray(a, dtype=np.float32),
                                dtype=ml_dtypes.bfloat16)


def _host_consts():
    x = np.arange(64)
    kx = np.arange(64)
    c = np.arange(32)
    y = np.arange(64)
    X = np.arange(128)
    Y = np.arange(128)

    FRs = np.where(kx <= 32, kx, kx - 64).astype(np.float64)

    ExR = np.cos(2 * np.pi * np.outer(kx, x) / 64)
    ExI = -np.sin(2 * np.pi * np.outer(kx, x) / 64)
    ExF = np.concatenate([ExR.T, ExI.T, -ExR.T], axis=1)     # [x, 192]

    EyCT = np.cos(2 * np.pi * np.outer(c, y) / 64).T          # [y, 32]
    EyST = np.sin(2 * np.pi * np.outer(c, y) / 64).T

    S_sel = np.zeros((128, 32))
    for im in range(4):
        S_sel[im * 32 + np.arange(32), np.arange(32)] = 1.0

    den = FRs[None, :] ** 2 + c[:, None].astype(np.float64) ** 2
    den[0, 0] = 1.0
    t_u = c[:, None] / den                            # [32, 64]
    s_v = -FRs[None, :] / den
    t_rep = np.tile(t_u, (1, 8))                      # [32, 512]
    s_rep = np.tile(s_v, (1, 8))
    tsg = np.concatenate([-t_rep, t_rep, -s_rep, s_rep], axis=1)  # [32, 2048]

    w_c = np.where(c == 0, 1.0, 2.0)
    s_q = 2.0 / (128.0 * 128.0)
    QRT = (s_q * w_c[None, :] * np.cos(2 * np.pi * np.outer(Y, c) / 128)).T
    QIT = (s_q * w_c[None, :] * np.sin(2 * np.pi * np.outer(Y, c) / 128)).T
    QFRs = np.concatenate([QRT, -QIT], axis=0)        # [64, 128] K-stack
    QFIs = np.concatenate([QIT, QRT], axis=0)

    PRT = np.cos(2 * np.pi * np.outer(FRs, X) / 128)  # [64, 128]
    PIT = np.sin(2 * np.pi * np.outer(FRs, X) / 128)
    PRT[32, :] = 0.0
    PIT[32, :] = 0.0
    PRTPnIT = np.concatenate([PRT, -PIT], axis=0)     # [128, 128] K-stack

    # fr direct path
    EyRm = np.cos(2 * np.pi * np.outer(c, y) / 64)
    EyIm = -np.sin(2 * np.pi * np.outer(c, y) / 64)
    QRm = s_q * w_c[None, :] * np.cos(2 * np.pi * np.outer(Y, c) / 128)
    QIm = s_q * w_c[None, :] * np.sin(2 * np.pi * np.outer(Y, c) / 128)
    Rx = PRT.T @ ExR - PIT.T @ ExI                    # [128, 64]
    Cy = QRm @ EyRm - QIm @ EyIm                      # [128, 64]
    RxT = Rx.T                                        # [64, 128]
    CyT = np.concatenate([Cy.T, Cy.T], axis=0)        # [128, 128]

    dup = lambda m: np.concatenate([m, m], axis=0)   # both partition halves
    return dict(ExF=_bf16(ExF), EyCT=_bf16(dup(EyCT)), EyST=_bf16(dup(EyST)),
                S_sel=_bf16(S_sel), tsg=_bf16(tsg), QF1=_bf16(QF1),
                QF2=_bf16(QF2), PRT64=_bf16(dup(PRT)), nPIT64=_bf16(dup(nPIT)),
                RxT=_bf16(RxT), CyT=_bf16(CyT))


def _rot90_kernel(k):
    yk = np.swapaxes(k, -2, -1)
    return np.concatenate([yk[..., :1], yk[..., :0:-1]], axis=-1)


def _symmetric_kernel(k):
    k1 = k
    k2 = _rot90_kernel(k1)
    k3 = _rot90_kernel(k2)
    k4 = _rot90_kernel(k3)
    k5 = np.swapaxes(k1, -2, -1)
    k6 = _rot90_kernel(k5)
    k7 = _rot90_kernel(k6)
    k8 = _rot90_kernel(k7)
    return (k1 + k2 + k3 + k4 + k5 + k6 + k7 + k8) / 8.0


def _prep_k_all(kernel_np):
    """kernel [1,8,16,64,64] -> k_all [128, 2048] conv-layout, bf16."""
    ksym = _symmetric_kernel(kernel_np.astype(np.float64))[0]
    K = np.fft.rfft2(ksym).real                                 # [8,16,64,33]
    Kc = np.transpose(K[:, :, :, :32], (0, 1, 3, 2)).copy()     # [i,j,c,kx]
    Kc[:, :, :, 32] = 0.0
    k_all = np.zeros((128, 2048), dtype=np.float64)
    for i in range(8):
        h, im = i // 4, i % 4
        for j in range(16):
            k_all[im * 32:(im + 1) * 32,
                  j * 128 + h * 64: j * 128 + h * 64 + 64] = Kc[i, j]
    return _bf16(k_all)


def _bcast(ap, n, axis_pos=1):
    dims = list(ap.ap)
    dims.insert(axis_pos, [0, n])
    return AP(ap.tensor, ap.offset, dims)


def _view(ap, offset_elems, dims):
    return AP(ap.tensor, ap.offset + offset_elems, dims)


DEFAULT_CFG = dict(
    d_at='act', d_t1='act', d_fcv='act', d_acv='act', d_g='act',
    d_psu='act', d_fr='act',
    d_at0='dve', d_t10='dve', d_fcv0='dve',
    acv_direct=True, fr_late=True, sample_major=True,
    mw_eng=None, bubv_eng=None,
    tt_pool_frac=0.48,       # relative weight of Pool in the TT split
    dma_chunks=(('sp', 'act') * 3, ('act', 'sp') * 3),
)


def build_program(reps=1, **cfg_over):
    cfg = dict(DEFAULT_CFG)
    cfg.update(cfg_over)
    nc = bacc.Bacc("TRN2", target_bir_lowering=False)
    consts = _host_consts()

    f_in = nc.dram_tensor("f_in", [B_PER_CORE, C1, 64, 64], F32,
                          kind="ExternalInput")
    k_in = nc.dram_tensor("k_all", [128, 2048], BF16, kind="ExternalInput")
    out_sh = nc.dram_tensor("out_sh", [B_PER_CORE, 128, NCH_OUT, 128], BF16,
                            kind="ExternalOutput")

    cdr = {n: nc.inline_tensor(a, name=f"c_{n}") for n, a in consts.items()}

    mix_tick = [0]

    def drain(which, out_ap, in_ap):
        e = cfg[which]
        if e == 'mix':
            mix_tick[0] += 1
            e = 'dve' if mix_tick[0] % 2 else 'act'
        if e == 'dve':
            nc.vector.tensor_copy(out_ap, in_ap)
        else:
            nc.scalar.copy(out=out_ap, in_=in_ap)

    # weighted greedy balance of TT ops between DVE and Pool
    tt_state = [0.0, 0.0]       # projected ns on dve, pool

    def tt_eng(fe, pref=None):
        if pref == 'dve':
            tt_state[0] += fe * 0.521 + 60
            return nc.vector
        if pref == 'gps':
            tt_state[1] += fe * 0.833 + 25
            return nc.gpsimd
        w_pool = cfg['tt_pool_frac']
        t_d = (tt_state[0] + fe * 0.521 + 60) / max(1.0 - w_pool, 1e-6)
        t_p = (tt_state[1] + fe * 0.833 + 25) / max(w_pool, 1e-6)
        if t_p < t_d:
            tt_state[1] += fe * 0.833 + 25
            return nc.gpsimd
        tt_state[0] += fe * 0.521 + 60
        return nc.vector

    with tile.TileContext(nc) as tc:
        with (
            tc.tile_pool(name="cp", bufs=1) as cp,
            tc.tile_pool(name="wk", bufs=2) as wk,
            tc.tile_pool(name="uv", bufs=2) as uvp,
            tc.tile_pool(name="wp", bufs=1) as wp,
            tc.tile_pool(name="stp", bufs=2) as stp,
            tc.tile_pool(name="pp", bufs=1, space="PSUM") as pp,
        ):
            # ---- loads: f first (needed earliest), consts spread ----
            st = {b: {} for b in range(B_PER_CORE)}
            for b in range(B_PER_CORE):
                fsb32 = wk.tile([64, 512], F32, tag="fsb32", name="fsb32")
                nc.sync.dma_start(
                    out=fsb32[:].rearrange("x (i y) -> x i y", i=8),
                    in_=f_in[b].rearrange("i x y -> x i y"))
                fsb = wk.tile([64, 512], BF16, tag="fsb", name="fsb")
                (nc.gpsimd if cfg.get('fsb_gps') else
                 nc.vector).tensor_copy(fsb[:], fsb32[:])
                st[b]['fsb'] = fsb

            cs = {}
            lq = [nc.sync, nc.scalar]
            order = ["ExF", "RxT", "EyCT", "EyST", "S_sel", "CyT",
                     "tsg", "QF1", "QF2", "PRT64", "nPIT64"]
            for li, name in enumerate(order):
                arr = consts[name]
                t = cp.tile(list(arr.shape), BF16, tag=f"c_{name}",
                            name=f"cs_{name}")
                lq[li % 2].dma_start(out=t[:], in_=cdr[name][:])
                cs[name] = t
            k_sb = cp.tile([128, 2048], BF16, tag="k_sb", name="k_sb")
            nc.scalar.dma_start(out=k_sb[:], in_=k_in[:])

            qmap = {'sp': nc.sync, 'act': nc.scalar, 'gps': nc.gpsimd}

            def emit_stage1(b):
                s = st[b]
                fsb = s['fsb']
                a_ts, t1s = [], []
                for ip2 in range(2):
                    psA = pp.tile([128, 384], F32, tag="bankA", bufs=2,
                                  name="psA")
                    psT1 = pp.tile([64, 512], F32, tag="bankA", bufs=2,
                                   name="psT1")
                    for ipl in range(2):
                        ip = 2 * ip2 + ipl
                        lhs = fsb[:, ip * 128:(ip + 1) * 128]
                        nc.tensor.matmul(psA[:, ipl * 192:(ipl + 1) * 192],
                                         lhs, cs["ExF"][:],
                                         start=True, stop=True)
                    for k in range(4):
                        ch = 4 * ip2 + k
                        nc.tensor.matmul(psT1[:, k * 128:(k + 1) * 128],
                                         fsb[:, ch * 64:(ch + 1) * 64],
                                         cs["RxT"][:],
                                         start=True, stop=True)
                    a_t = wk.tile([128, 384], BF16, tag=f"at{ip2}",
                                  name=f"at{ip2}")
                    drain('d_at', a_t[:], psA[:], b)
                    a_ts.append(a_t)
                    t1 = wk.tile([64, 512], BF16, tag=f"t1{ip2}",
                                 name=f"t1{ip2}")
                    drain('d_t1', t1[:], psT1[:], b)
                    t1s.append(t1)
                s['a_ts'] = a_ts
                s['t1s'] = t1s

            def emit_stage2(b):
                s = st[b]
                psF = [pp.tile([128, 128], F32, tag=f"bankF{h}",
                               name=f"psF{h}") for h in range(2)]
                for i in range(8):
                    a_t = s['a_ts'][i // 4]
                    base = ((i // 2) % 2) * 192
                    po = (i % 2) * 64
                    A_RI = a_t[po:po + 64, base:base + 128]
                    A_IS = a_t[po:po + 64, base + 64:base + 192]
                    h, im = i // 4, i % 4
                    sl = slice(im * 32, (im + 1) * 32)
                    tp = (po, im * 32)
                    nc.tensor.matmul(psF[h][sl, :],
                                     cs["EyCT"][po:po + 64, :], A_RI,
                                     start=True, stop=False, tile_position=tp)
                    nc.tensor.matmul(psF[h][sl, :],
                                     cs["EyST"][po:po + 64, :], A_IS,
                                     start=False, stop=True, tile_position=tp)
                Fcv = wk.tile([128, 256], BF16, tag="Fcv", name="Fcv")
                for h in range(2):
                    drain('d_fcv', Fcv[:, h * 128:(h + 1) * 128], psF[h][:])
                s['Fcv'] = Fcv

            def emit_conv(b):
                s = st[b]
                Fcv = s['Fcv']
                Mw = []
                for RI in range(2):
                    m_t = wp.tile([128, 2048], BF16, tag=f"mw{RI}", bufs=2,
                                  name=f"mw{RI}")
                    in0 = _view(Fcv[:], RI * 64,
                                [Fcv[:].ap[0], [0, 16], [128, 2], [1, 64]])
                    tt_eng(2048, cfg['mw_eng']).tensor_mul(
                        m_t[:].rearrange("p (j h f) -> p j h f", j=16, h=2),
                        in0,
                        k_sb[:].rearrange("p (j h f) -> p j h f", j=16, h=2))
                    Mw.append(m_t)

                Bu = wk.tile([64, 1024], BF16, tag="Bu", name="Bu")
                Bv = wk.tile([64, 1024], BF16, tag="Bv", name="Bv")
                BuR, BuI = Bu[0:32, :], Bu[32:64, :]
                BvR, BvI = Bv[0:32, :], Bv[32:64, :]
                tsg = cs["tsg"]
                for RI in range(2):
                    for jh in range(2):
                        ps_acv = pp.tile([32, 512], F32, tag="bankA", bufs=2,
                                         name="ps_acv")
                        for h in range(2):
                            rhs = _view(Mw[RI][:], jh * 1024 + h * 64,
                                        [Mw[RI][:].ap[0], [128, 8], [1, 64]])
                            nc.tensor.matmul(ps_acv[:], cs["S_sel"][:], rhs,
                                             start=(h == 0), stop=(h == 1))
                        if cfg.get('acv_direct'):
                            tt_state[0] += 2 * (512 * 1.0417 + 60)
                            beng0 = beng1 = nc.vector
                            src_ap = ps_acv[:]
                        else:
                            acv = wk.tile([32, 512], BF16, tag="acv",
                                          name="acv")
                            drain('d_acv', acv[:], ps_acv[:], b)
                            src_ap = acv[:]
                            beng0 = tt_eng(512, cfg['bubv_eng'])
                            beng1 = tt_eng(512, cfg['bubv_eng'])
                        osl = slice(jh * 512, (jh + 1) * 512)
                        if RI == 0:
                            beng0.tensor_mul(BuI[:, osl], src_ap,
                                             tsg[:, 512:1024])
                            beng1.tensor_mul(BvI[:, osl], src_ap,
                                             tsg[:, 1536:2048])
                        else:
                            beng0.tensor_mul(BuR[:, osl], src_ap,
                                             tsg[:, 0:512])
                            beng1.tensor_mul(BvR[:, osl], src_ap,
                                             tsg[:, 1024:1536])
                s['B'] = (Bu, Bv)

            def emit_staging(b):
                s = st[b]
                sts = []
                for ci in range(len(CH_SPLITS) - 1):
                    ncols = (CH_SPLITS[ci + 1] - CH_SPLITS[ci]) * 128
                    stt = stp.tile([128, ncols], BF16, tag=f"st{ci}",
                                   name=f"st{ci}")
                    if cfg.get('level', 99) < 4:
                        nc.vector.memset(stt[:], 0.0)
                    sts.append(stt)
                s['sts'] = sts

            def emit_fr(b):
                s = st[b]
                sts = s['sts']
                for ip2 in range(2):
                    psUf = pp.tile([128, 512], F32, tag=f"bankF{2 + ip2}",
                                   name="psUf")
                    for k in range(4):
                        nc.tensor.matmul(
                            psUf[:, k * 128:(k + 1) * 128],
                            s['t1s'][ip2][0:64, k * 128:(k + 1) * 128],
                            cs["CyT"][0:64, :],
                            start=True, stop=True)
                    drain('d_fr', sts[0][:, ip2 * 512:(ip2 + 1) * 512],
                          psUf[:], b)

            ps_tick = [0]

            def emit_synth(b):
                s = st[b]
                Bu, Bv = s['B']
                u_q, v_q = [], []
                for q in range(4):
                    for B_, dst_list in ((Bu, u_q), (Bv, v_q)):
                        ps_tick[0] += 1
                        psG = pp.tile([128, 512], F32,
                                      tag=f"bankF{2 + ps_tick[0] % 2}",
                                      name="psG")
                        for chl in range(4):
                            ch = q * 4 + chl
                            lhsT = B_[0:64, ch * 64:(ch + 1) * 64]
                            ccol = slice(chl * 128, (chl + 1) * 128)
                            nc.tensor.matmul(
                                psG[0:64, ccol], lhsT, cs["QFRs"],
                                start=True, stop=True,
                                tile_position=(0, 0))
                            nc.tensor.matmul(
                                psG[64:128, ccol], lhsT, cs["QFIs"],
                                start=True, stop=True,
                                tile_position=(0, 64))
                        gsb = wk.tile([128, 512], BF16, tag="gsb",
                                      name="gsb", bufs=cfg.get("gbufs", 3))
                        drain('d_g', gsb[:], psG[:], b)
                        psU = pp.tile([128, 512], F32,
                                      tag=f"bankF{4 + ps_tick[0] % 2}",
                                      name="psU")
                        nc.tensor.matmul(psU[:], cs["PRTPnIT"], gsb[:],
                                         start=True, stop=True)
                        nm = ('u' if dst_list is u_q else 'v') + f"q{q}"
                        qt = uvp.tile([128, 512], BF16, tag=nm, name=nm)
                        drain('d_psu', qt[:], psU[:], b)
                        dst_list.append(qt)
                s['u_q'] = u_q
                s['v_q'] = v_q

            def emit_cross(b):
                s = st[b]
                u_q, v_q, sts = s['u_q'], s['v_q'], s['sts']

                def st_sub(a, b0, cnt, in0, in1):
                    pch = PAIR_BASE[a] + (b0 - a - 1)
                    ci = max(i for i in range(len(CH_SPLITS) - 1)
                             if CH_SPLITS[i] <= pch)
                    assert pch + cnt <= CH_SPLITS[ci + 1], (a, b0, cnt)
                    out = sts[ci][:, (pch - CH_SPLITS[ci]) * 128:
                                  (pch - CH_SPLITS[ci] + cnt) * 128]
                    tt_eng(cnt * 128).tensor_sub(
                        out.rearrange("p (c y) -> p c y", c=cnt), in0, in1)

                def prod_block(W, uq, vq):
                    # W[p, a, b, y] = u_a * v_b in ONE instruction
                    out = W[:].rearrange("p (a b y) -> p a b y", a=4, b=4)
                    in0 = _view(uq[:], 0,
                                [uq[:].ap[0], [128, 4], [0, 4], [1, 128]])
                    in1 = _view(vq[:], 0,
                                [vq[:].ap[0], [0, 4], [128, 4], [1, 128]])
                    tt_eng(2048).tensor_mul(out, in0, in1)

                for gI in range(4):
                    for gJ in range(gI, 4):
                        W1 = wp.tile([128, 2048], BF16, tag="W1",
                                     bufs=cfg.get("wbufs", 3),
                                     name="W1")
                        prod_block(W1, u_q[gI], v_q[gJ])
                        if gI != gJ:
                            W2 = wp.tile([128, 2048], BF16, tag="W2",
                                         bufs=cfg.get("wbufs", 3),
                                         name="W2")
                            prod_block(W2, u_q[gJ], v_q[gI])
                            for ai in range(4):
                                a = 4 * gI + ai
                                in0 = _view(W1[:], ai * 512,
                                            [W1[:].ap[0], [128, 4], [1, 128]])
                                in1 = _view(W2[:], ai * 128,
                                            [W2[:].ap[0], [512, 4], [1, 128]])
                                st_sub(a, 4 * gJ, 4, in0, in1)
                        else:
                            for ai in range(3):
                                a = 4 * gI + ai
                                cnt = 3 - ai
                                in0 = _view(W1[:], ai * 512 + (ai + 1) * 128,
                                            [W1[:].ap[0], [128, cnt],
                                             [1, 128]])
                                in1 = _view(W1[:], (ai + 1) * 512 + ai * 128,
                                            [W1[:].ap[0], [512, cnt],
                                             [1, 128]])
                                st_sub(a, a + 1, cnt, in0, in1)

            def emit_dma(b):
                s = st[b]
                for ci in range(len(CH_SPLITS) - 1):
                    c0, c1 = CH_SPLITS[ci], CH_SPLITS[ci + 1]
                    eng = qmap[cfg['dma_chunks'][b][ci]]
                    eng.dma_start(
                        out=out_sh[b, :, c0:c1, :],
                        in_=s['sts'][ci][:].rearrange("p (c y) -> p c y",
                                                      c=c1 - c0))

            lvl = cfg.get('level', 99)
            phases = [emit_staging, emit_stage1, emit_stage2]
            mid = []
            if lvl >= 1:
                mid.append(emit_conv)
            if cfg.get('fr_late'):
                if lvl >= 3:
                    mid.append(emit_synth)
                if lvl >= 2:
                    mid.append(emit_fr)
            else:
                if lvl >= 2:
                    mid.append(emit_fr)
                if lvl >= 3:
                    mid.append(emit_synth)
            for rep in range(reps):
                for ph in phases:
                    for b in range(B_PER_CORE):
                        ph(b)
                if cfg.get('sample_major'):
                    for b in range(B_PER_CORE):
                        for ph in mid:
                            ph(b)
                else:
                    for ph in mid:
                        for b in range(B_PER_CORE):
                            ph(b)
                for b in range(B_PER_CORE):
                    if lvl >= 4:
                        emit_cross(b)
                    emit_dma(b)
    nc.compile()
    return nc


_PROGRAM = {}


def _get_program(reps=1, **kw):
    key = (reps, tuple(sorted(kw.items())))
    if key not in _PROGRAM:
        _PROGRAM[key] = build_program(reps, **kw)
    return _PROGRAM[key]


LAST_EXEC_NS = None
LAST_RESULT = None


def kernel(f, kernel):
    global LAST_EXEC_NS, LAST_RESULT
    import os
    f = np.ascontiguousarray(f, dtype=np.float32)
    k_all = _prep_k_all(np.asarray(kernel))
    nc = _get_program()
    in_maps = [
        {"f_in": f[2 * c:2 * c + 2], "k_all": k_all} for c in range(N_CORES)
    ]
    trace = bool(os.environ.get("KERNEL_TRACE"))
    res = run_bass_kernel_spmd(nc, in_maps, list(range(N_CORES)), trace=trace)
    LAST_RESULT = res
    if res.exec_time_ns is not None:
        LAST_EXEC_NS = res.exec_time_ns
    out = np.concatenate(
        [np.asarray(res.results[c]["out_sh"]).astype(np.float32)
         for c in range(N_CORES)], axis=0)
    return out.transpose(0, 2, 1, 3)
